# revision 15
# baseline (speedup 1.0000x reference)
"""Trainium2 Bass kernel for nn_Graph_to_Featuremaps_savemem.

Reference computation:
    scores[b,p,n] = s_res[b,p] + s_hid[b,n];  attn = softmax_n(scores)
    out[b,c,p]    = relu(sum_n attn[b,p,n] * (x[b,n,:] @ W)[c])

Key simplification: softmax over n is shift-invariant, so the per-pixel
s_res[b,p] term (the only use of res_feature / node_fea_for_res) cancels:
    attn[b,p,n] = softmax_n(s_hid[b,n])   (independent of p)
    out[b,c,p]  = relu(sum_n a[b,n] * nv[b,n,c])  broadcast over all pixels.

So the kernel is a tiny softmax-weighted matmul (per-batch (7,256)x(256,256))
followed by a 151 MB broadcast-write of the (B,C) result over H*W pixels.
Sharding: data-parallel over batch, 2 batches per core across 8 cores; the
small params (node_fea_for_hidden, weight) are replicated.

Hardware constraints shaping the structure:
- PE matmul / tensor-scalar / DMA-trigger instructions have a single
  sync-wait slot, so every PE operand pair must share one producer
  semaphore. All small inputs (w, x, nfh, identity, block-mask, ones) are
  packed host-side into ONE DRAM tensor loaded by ONE DMA; PSUM results are
  funneled through DVE copies.
- The kernel-tail drain also has limited wait slots, so the kernel keeps the
  total semaphore count low: only ACT (which triggers all DMAs), PE, DVE and
  the 8 HW DMA queues are used.
- matmul operands need base partition 0/32/64; x and the transpose identity
  live at rows 32:46 of the packed tile, everything else at base 0.
"""

import numpy as np

import concourse.bass as bass
import concourse.mybir as mybir
import concourse.tile as tile
from concourse.bass_utils import run_bass_kernel_spmd

B, NODES, HID, C, H, W = 16, 7, 256, 256, 96, 96
P = H * W                # 9216 pixels
NCORES = 8
BL = B // NCORES         # 2 local batches per core
BN = BL * NODES          # 14 (b,n) rows
WCHUNK = 9216            # broadcast tile width; P = 1 * WCHUNK
NCHUNK = P // WCHUNK

# Packed input layout: (128, CIN_COLS) float32
COL_W = 0        # cols 0:512, all rows: w[kh*128+k, c] at [k, kh*256+c]
COL_ID = 512     # cols 512:526, rows 32:46: identity(14)
COL_BM = 526     # cols 526:528, rows 0:14: block-diagonal mask (14, 2)
COL_XN = 528     # cols 528:784: row 0 = nfh; rows 32:46 = x[(b n), h]
COL_ONE = 784    # col 784, row 0: 1.0
CIN_COLS = 785
XROW = 32        # base partition for x / identity (must be 0, 32 or 64)

_cache: dict = {}


def _build_nc():
    nc = bass.Bass()
    dt = mybir.dt.float32
    cin_d = nc.declare_dram_parameter("cin", [128, CIN_COLS], dt, isOutput=False)
    out_d = nc.declare_dram_parameter("out", [BL, C, P], dt, isOutput=True)

    with tile.TileContext(nc) as tc:
        with (
            tc.tile_pool(name="sb", bufs=1) as sb,
            tc.tile_pool(name="ps", bufs=1, space=bass.MemorySpace.PSUM) as ps,
        ):
            cin = sb.tile([128, CIN_COLS], dt)
            nc.scalar.dma_start(out=cin[:], in_=cin_d[:])
            x_sl = cin[XROW : XROW + BN, COL_XN : COL_XN + HID]
            ident = cin[XROW : XROW + BN, COL_ID : COL_ID + BN]
            nfh_row = cin[0:1, COL_XN : COL_XN + HID]
            one_cin = cin[0:1, COL_ONE : COL_ONE + 1]

            ones11 = sb.tile([1, 1], dt)
            nc.vector.memset(ones11[:], 1.0)
            sb_w = sb.tile([128, 2 * C], dt)
            nc.vector.tensor_copy(out=sb_w[:], in_=cin[:, 0 : 2 * C])
            blkmask = sb.tile([BN, BL], dt)
            nc.vector.tensor_copy(out=blkmask[:], in_=cin[0:BN, COL_BM : COL_BM + BL])

            # PE-transpose x to (h, bn) layout, one (128, 14) tile per k-half.
            sbT = []
            for kh in range(2):
                p_t = ps.tile([128, BN], dt, tag=f"xT{kh}")
                nc.tensor.transpose(p_t[:], x_sl[:, kh * 128 : (kh + 1) * 128], ident)
                s_t = sb.tile([128, BN], dt, tag=f"sbT{kh}")
                nc.vector.tensor_copy(out=s_t[:], in_=p_t[:])
                sbT.append(s_t)
            # Transpose nfh row to a (128, kh) column pair via K=1 matmuls.
            p_nfh = ps.tile([128, 2], dt, tag="nfhT")
            for kh in range(2):
                nc.tensor.matmul(
                    p_nfh[:, kh : kh + 1],
                    nfh_row[:, kh * 128 : (kh + 1) * 128],
                    one_cin,
                    start=True,
                    stop=True,
                )
            sb_nfh_col = sb.tile([128, 2], dt)
            nc.vector.tensor_copy(out=sb_nfh_col[:], in_=p_nfh[:])

            # s_hid row (1, 14) and node_vals (14, 256), contracting h in 2 halves.
            ps_s = ps.tile([1, BN], dt, tag="s")
            ps_nv = ps.tile([BN, C], dt, tag="nv")
            for kh in range(2):
                nc.tensor.matmul(
                    ps_s[:],
                    sb_nfh_col[:, kh : kh + 1],
                    sbT[kh][:],
                    start=(kh == 0),
                    stop=(kh == 1),
                )
                nc.tensor.matmul(
                    ps_nv[:],
                    sbT[kh][:],
                    sb_w[:, kh * C : (kh + 1) * C],
                    start=(kh == 0),
                    stop=(kh == 1),
                )
            sb_nv = sb.tile([BN, C], dt)
            nc.vector.tensor_copy(out=sb_nv[:], in_=ps_nv[:])

            # Softmax over the 7 nodes (free dim), separately per local batch.
            e_row = sb.tile([1, BN], dt)
            denom = sb.tile([1, BL], dt)
            recip = sb.tile([1, BL], dt)
            a_row = sb.tile([1, BN], dt)
            for b in range(BL):
                nc.scalar.activation(
                    e_row[:, b * NODES : (b + 1) * NODES],
                    ps_s[:, b * NODES : (b + 1) * NODES],
                    mybir.ActivationFunctionType.Exp,
                    accum_out=denom[:, b : b + 1],
                )
            nc.vector.reciprocal(recip[:], denom[:])
            for b in range(BL):
                nc.vector.tensor_scalar_mul(
                    a_row[:, b * NODES : (b + 1) * NODES],
                    e_row[:, b * NODES : (b + 1) * NODES],
                    recip[:, b : b + 1],
                )

            # Transpose attn row to a column via K=1 matmul: ps_a[(b,n), 0] = a[b, n],
            # then expand into a block-diagonal (14, BL) matrix so one matmul per
            # c-half computes v for both local batches.
            ps_a = ps.tile([BN, 1], dt, tag="a")
            nc.tensor.matmul(ps_a[:], a_row[:], ones11[:], start=True, stop=True)
            sb_a = sb.tile([BN, 1], dt)
            nc.vector.tensor_copy(out=sb_a[:], in_=ps_a[:])
            rhs_a = sb.tile([BN, BL], dt)
            nc.vector.tensor_scalar_mul(rhs_a[:], blkmask[:], sb_a[:])

            # v[c, (ch, b)] = sum_n a[b, n] * nv[(b,n), c]; relu; broadcast; store.
            ps_v = ps.tile([128, 2 * BL], dt, tag="v")
            for ch in range(2):
                nc.tensor.matmul(
                    ps_v[:, ch * BL : (ch + 1) * BL],
                    sb_nv[:, ch * 128 : (ch + 1) * 128],
                    rhs_a[:],
                    start=True,
                    stop=True,
                )
            sb_v = sb.tile([128, 2 * BL], dt)
            nc.scalar.activation(sb_v[:], ps_v[:], mybir.ActivationFunctionType.Relu)
            # One broadcast tile and ONE output DMA for the whole shard: the full
            # out (BL, 256, P) viewed as [p, b, ch, pix] with c = ch*128 + p.
            # A single DMA keeps the DMA-queue semaphore count minimal (the
            # kernel-tail drain has very few wait slots on this walrus).
            bc = sb.tile([128, BL, 2, P], dt)
            for b in range(BL):
                for ch in range(2):
                    j = ch * BL + b
                    nc.vector.tensor_copy(
                        out=bc[:, b, ch, :], in_=sb_v[:, j : j + 1].to_broadcast([128, P])
                    )
            nc.scalar.dma_start(
                out=out_d.rearrange("b (ch p) pix -> p b ch pix", p=128),
                in_=bc[:],
            )
    _fix_tail_drain(nc)
    return nc


def _fix_tail_drain(nc):
    """Walrus in this toolchain accepts very few sync waits per instruction, and
    Tile's kernel-tail drain waits on every semaphore. In this kernel the whole
    dataflow is one chain ending in the single output DMA: every other sem tick
    (input-DMA queue, PE, DVE, ACT) is strictly upstream of the output-DMA
    trigger, so waiting on the output queue's completion sem alone is
    sufficient. Strip the drain down to that one wait."""
    import bass_rust

    out_sem = None
    for ins in nc.inst_map.values():
        if type(ins).__name__ == "InstDMACopy" and "out_set" in str(ins):
            si = ins.sync_info
            if si is not None and len(si.on_update) > 0:
                out_sem = si.on_update[0].ant_name
    assert out_sem is not None, "output DMA completion sem not found"
    for ins in nc.inst_map.values():
        si = ins.sync_info
        if type(ins).__name__ == "InstDrain" and si is not None and len(si.on_wait) > 1:
            keep = [w for w in si.on_wait if w.ant_name == out_sem]
            assert len(keep) == 1, (out_sem, [w.ant_name for w in si.on_wait])
            ins.sync_info = bass_rust.SyncInfo(
                on_wait=keep, on_update=list(si.on_update)
            )


def _get_nc():
    if "nc" not in _cache:
        _cache["nc"] = _build_nc()
    return _cache["nc"]


def _pack_cin(x_shard, nfh, w):
    """Pack one core's inputs into the (128, CIN_COLS) tensor."""
    cin = np.zeros((128, CIN_COLS), dtype=np.float32)
    # w: [kh*128+k, c] -> [k, kh*256+c]
    cin[:, 0:C] = w[0:128, :]
    cin[:, C : 2 * C] = w[128:256, :]
    cin[XROW : XROW + BN, COL_ID : COL_ID + BN] = np.eye(BN, dtype=np.float32)
    for b in range(BL):
        cin[b * NODES : (b + 1) * NODES, COL_BM + b] = 1.0
    cin[0, COL_XN : COL_XN + HID] = nfh[:, 0]
    cin[XROW : XROW + BN, COL_XN : COL_XN + HID] = x_shard.reshape(BN, HID)
    cin[0, COL_ONE] = 1.0
    return cin


def _make_in_maps(input, node_fea_for_hidden, weight):
    x_full = np.asarray(input, dtype=np.float32)[0]  # (B, N, HID)
    nfh = np.asarray(node_fea_for_hidden, dtype=np.float32)
    w = np.asarray(weight, dtype=np.float32)
    return [
        {"cin": _pack_cin(x_full[i * BL : (i + 1) * BL], nfh, w)}
        for i in range(NCORES)
    ]


def _run(in_maps, trace=False, **kwargs):
    nc = _get_nc()
    return run_bass_kernel_spmd(nc, in_maps, list(range(NCORES)), trace=trace, **kwargs)


def kernel(input, res_feature, node_fea_for_res, node_fea_for_hidden, weight):
    in_maps = _make_in_maps(input, node_fea_for_hidden, weight)
    res = _run(in_maps)
    shards = [res.results[i]["out"] for i in range(NCORES)]  # each (BL, C, P)
    full = np.concatenate(shards, axis=0)  # (B, C, P)
    return full.reshape(B, C, H, W).astype(np.float32, copy=False)
